# revision 14
# baseline (speedup 1.0000x reference)
"""Trainium2 Bass kernel for nn_DeepSCTransformerBlock.

Sharding: 8 cores = 4 batches x 2 branches (gene/expr). Zero collectives.
One SPMD program; the gene branch is expressed through the expr skeleton by
host-side weight folding, and the (s2, d1) normalization coefficients
(cA, cB) are per-core input data:
  gene: scale_i = 1/(1*s2 + 1e-8*d1)   [signed-L1 renorm of softmax*mask]
  expr: scale_i = 1/(0*s2 + 1*d1)      [plain softmax denominator]
where E = exp(S), P = E*M, d1 = colsum(E), s2 = colsum(P); O = scale * (P @ V).

The denominator cA*s2 + cB*d1 is produced directly in PSUM row 64 of the AV
accumulator: V carries an extra column equal to cA (so the AV matmul adds
cA * colsum(P)), and an interleaved k=1 matmul with a cB-filled stationary
accumulates cB * colsum(E) into the same row.

Host-side folds (all exact, fp32):
  - fused head projection: Qf = concat(Q1,Q2) @ fwq + fbq collapses into two
    D x D weights (per-head block product) -> one accumulated projection.
  - LayerNorm affine (gamma/beta) folded into every consumer weight/bias, so
    on-chip LN is just (x - mean) * rstd.
  - out-proj bias bo folded into the residual input (in2b = in2 + bo).
  - M passed pre-transposed ([j, i]) so the mask tile is a straight DMA.

All matmul operands are bf16 (1 PE cycle/row vs 4 for fp32); PSUM accumulates
fp32; the residual stream, LN statistics, and softmax scale math stay fp32.
Activations flow feature-major ([d, tokens]) through the matmul chains; S is
computed transposed ([j, i]) so the AV contraction needs no transposes of the
attention matrix; FFN2 contracts feature-major h1 directly into token-major
output (no final transposes). Bulk weight prefetches ride the gpsimd DMA
queue so they never block the input loads on the sync queue.
"""

import numpy as np

B, G, D, H = 4, 1024, 512, 8
HD = D // H
DFF = 4 * D
P = 128
ICH = G // P      # 8 token chunks
KC = D // P       # 4 feature chunks
FC = DFF // P     # 16 ffn-hidden chunks
NIH = 2           # i halves (free-dim 512 per matmul)
IH = G // NIH     # 512
SCALE = 1.0 / (HD ** 0.5)
EPS = 1e-5

_cache = {}


def _build_program(split_waits=True):
    import contextlib
    import concourse.bass as bass
    import concourse.mybir as mybir
    import concourse.tile as tile
    from concourse.masks import make_identity

    # walrus CoreV3 codegen rejects instructions carrying >1 sem wait at the
    # Tile end-of-kernel drain; split the waits across single-wait nops.
    def _patched_drain_and_barrier(self, tick_clock, wait_clock):
        nc = self.nc
        drain_inst = nc.sync.drain()
        wait_clock.add_sem_waits(
            drain_inst.ins, tile.ScopedClock({None: tick_clock.global_clock})
        )
        si = drain_inst.ins.sync_info
        if si is not None and si.on_wait and len(si.on_wait) > 1:
            waits = list(si.on_wait)
            si.on_wait = waits[:1]
            for i in range(1, len(waits)):
                nop = nc.sync.nop(hint="drain_wait_split", nofuse=True)
                nop.ins.sync_info = mybir.SyncInfo(
                    on_wait=waits[i : i + 1], on_update=[]
                )
        nc.all_engine_barrier()
        assert self.sems is not None
        popped = nc._tile_sem_poison_stack.pop()
        assert popped is self._sem_poison
        nc.clear_and_free_semaphores(list(self.sems.allocated().values()))
        nc.all_engine_barrier()

    tile.TileContext._drain_and_barrier = _patched_drain_and_barrier

    f32 = mybir.dt.float32
    bf16 = mybir.dt.bfloat16
    AF = mybir.ActivationFunctionType
    OP = mybir.AluOpType

    nc = bass.Bass()

    def dram_in(name, shape, dt=f32):
        return nc.dram_tensor(name, list(shape), dt, kind="ExternalInput")

    in1 = dram_in("in1", (G, D))
    in2 = dram_in("in2", (G, D))
    in2b = dram_in("in2b", (G, D))           # in2 + bo (residual base)
    MdT = dram_in("MT", (G, G), bf16)        # pre-transposed mask [j, i]
    wcq1 = dram_in("wcq1", (D, D), bf16)
    wcq2 = dram_in("wcq2", (D, D), bf16)
    wck1 = dram_in("wck1", (D, D), bf16)
    wck2 = dram_in("wck2", (D, D), bf16)
    fbq = dram_in("fbq", (D,))
    fbk = dram_in("fbk", (D,))
    wv2 = dram_in("wv2", (D, D), bf16); bv2 = dram_in("bv2", (D,))
    wo = dram_in("wo", (D, D), bf16)
    w1 = dram_in("w1", (D, DFF), bf16); b1 = dram_in("b1", (DFF,))
    w2 = dram_in("w2", (DFF, D), bf16); b2 = dram_in("b2", (D,))
    coef = dram_in("coef", (2,), bf16)       # [cA, cB]

    out_d = nc.dram_tensor("out", [G, D], f32, kind="ExternalOutput")

    with tile.TileContext(nc) as tc:
        with contextlib.ExitStack() as ctx:
            pc = ctx.enter_context(tc.tile_pool(name="const", bufs=1))
            identb = pc.tile([P, P], bf16, tag="identb")
            make_identity(nc, identb)
            eps_t = pc.tile([P, 1], f32, tag="eps")
            nc.vector.memset(eps_t, EPS)
            cB_col = pc.tile([P, 1], bf16, tag="cB_col")
            nc.sync.dma_start(out=cB_col, in_=coef[None, 1:2].to_broadcast([P, 1]))

            def rep_row(name, vec, n=D):  # [n] dram -> [P, n] replicated rows
                t = pc.tile([P, n], f32, tag=name)
                nc.gpsimd.dma_start(out=t, in_=vec[None, :].to_broadcast([P, n]))
                return t

            bv2_rep = rep_row("bv2_rep", bv2)
            b2_rep = rep_row("b2_rep", b2)

            def col(name, vec, n):  # [n*P] dram -> [P, n] column tile
                t = pc.tile([P, n], f32, tag=name)
                nc.gpsimd.dma_start(out=t, in_=vec.rearrange("(c p) -> p c", p=P))
                return t

            fbq_c = col("fbq_c", fbq, KC); fbk_c = col("fbk_c", fbk, KC)
            b1_c = col("b1_c", b1, FC)
            ones_row = pc.tile([1, HD], bf16, tag="ones_row")
            nc.vector.memset(ones_row, 1.0)

            # long-lived state: residual stream, mask, late weights.
            # bulk prefetches ride the gpsimd DMA queue so the sync queue
            # stays free for the phase-1 input chunk loads.
            pD = ctx.enter_context(tc.tile_pool(name="resid", bufs=1))
            OT = pD.tile([P, KC, G], bf16, tag="OT")        # merged heads, feature-major
            h_res = pD.tile([P, ICH, D], f32, tag="h_res")  # token-major residual
            MT = pD.tile([P, ICH, G], bf16, tag="MT")       # [j-part, jc, i]
            nc.gpsimd.dma_start(out=MT,
                                in_=MdT.rearrange("(jc p) i -> p jc i", p=P))
            wo_t = pD.tile([P, KC, D], bf16, tag="wo_t")
            nc.gpsimd.dma_start(out=wo_t, in_=wo.rearrange("(c p) n -> p c n", p=P))
            w1_t = pD.tile([P, KC, DFF], bf16, tag="w1_t")
            nc.gpsimd.dma_start(out=w1_t, in_=w1.rearrange("(c p) n -> p c n", p=P))
            w2_t = pD.tile([P, FC, D], bf16, tag="w2_t")
            nc.gpsimd.dma_start(out=w2_t, in_=w2.rearrange("(c p) n -> p c n", p=P))

            # long-lived attention operands (packed head layout: head h ->
            # partition rows (h%2)*64, chunk h//2); freed before phase 3
            pB_cm = tc.tile_pool(name="attn_ops", bufs=1)
            pB = pB_cm.__enter__()
            QfT = pB.tile([P, H // 2, G], bf16, tag="QfT")
            KfT = pB.tile([P, H // 2, G], bf16, tag="KfT")
            V_st = pB.tile([P, ICH, H, HD + 1], bf16, tag="V_st")  # [j, jc, h, 65]

            nc.vector.memset(V_st, 1.0)  # V parts overwritten below
            # stat columns <- cA  (AV matmul then adds cA * colsum(P) to row 64)
            nc.sync.dma_start(
                out=V_st.rearrange("p jc h c -> p (jc h) c")[:, :, HD:HD + 1],
                in_=coef[None, None, 0:1].to_broadcast([P, ICH * H, 1]))

            def ln_chunk(src_ap, xg_out, wkp):
                """Plain LayerNorm (no affine) of a [P, D] fp32 chunk -> bf16."""
                stats = wkp.tile([P, 6], f32, tag="ln_stats")
                mv = wkp.tile([P, 2], f32, tag="ln_mv")
                nc.vector.bn_stats(out=stats, in_=src_ap)
                nc.vector.bn_aggr(out=mv, in_=stats)
                stdt = wkp.tile([P, 1], f32, tag="ln_std")
                nc.scalar.activation(out=stdt, in_=mv[:, 1:2], func=AF.Sqrt,
                                     bias=eps_t, scale=1.0)
                rstd = wkp.tile([P, 1], f32, tag="ln_rstd")
                nc.vector.reciprocal(out=rstd, in_=stdt)
                nc.vector.tensor_scalar(out=xg_out, in0=src_ap, scalar1=mv[:, 0:1],
                                        scalar2=rstd, op0=OP.subtract, op1=OP.mult)

            # ============ phase 1: LN1, V proj, folded Qf/Kf proj ==========
            with tc.tile_pool(name="p1", bufs=1) as p1, \
                 tc.tile_pool(name="p1w", bufs=2) as p1w, \
                 tc.tile_pool(name="p1ps", bufs=2, space="PSUM") as p1ps, \
                 tc.tile_pool(name="p1qk", bufs=2, space="PSUM") as p1qk, \
                 tc.tile_pool(name="p1pt", bufs=2, space="PSUM") as p1pt:

                xgeT = p1.tile([P, KC, G], bf16, tag="xgeT")
                xeeT = p1.tile([P, KC, G], bf16, tag="xeeT")
                for (src, dstT) in ((in1, xgeT), (in2, xeeT)):
                    for ic in range(ICH):
                        xc = p1w.tile([P, D], f32, tag="ln_in")
                        nc.sync.dma_start(out=xc, in_=src[ic * P:(ic + 1) * P, :])
                        xg = p1w.tile([P, D], bf16, tag="ln_out")
                        ln_chunk(xc, xg, p1w)
                        pt = p1pt.tile([P, D], bf16, tag="pt")
                        for kc in range(KC):
                            nc.tensor.transpose(pt[:, kc * P:(kc + 1) * P],
                                                xg[:, kc * P:(kc + 1) * P], identb)
                        nc.scalar.activation(
                            out=dstT[:, :, ic * P:(ic + 1) * P],
                            in_=pt.rearrange("p (c i) -> p c i", i=P),
                            func=AF.Copy)

                def load_w(name, wd):
                    t = p1.tile([P, KC, D], bf16, tag=name)
                    nc.sync.dma_start(out=t, in_=wd.rearrange("(c p) n -> p c n", p=P))
                    return t

                wcq1_t = load_w("wcq1_t", wcq1)
                wcq2_t = load_w("wcq2_t", wcq2)
                wck1_t = load_w("wck1_t", wck1)
                wck2_t = load_w("wck2_t", wck2)
                wv2_t = load_w("wv2_t", wv2)

                # V projection (token-major) into V_st slots
                for jc in range(ICH):
                    ps = p1ps.tile([P, D], f32, tag="ps")
                    for kc in range(KC):
                        nc.tensor.matmul(ps,
                                         xeeT[:, kc, jc * P:(jc + 1) * P],
                                         wv2_t[:, kc, :],
                                         start=(kc == 0), stop=(kc == KC - 1))
                    nc.vector.tensor_tensor(
                        out=V_st[:, jc, :, 0:HD],
                        in0=ps.rearrange("p (h d) -> p h d", d=HD),
                        in1=bv2_rep.rearrange("p (h d) -> p h d", d=HD),
                        op=OP.add)

                # folded Qf/Kf projections, per head-pair chunk c:
                # QfT[:,c] = Wcq1[:,c]^T @ xgeT + Wcq2[:,c]^T @ xeeT + fbq
                for c in range(KC):
                    csl = slice(c * P, (c + 1) * P)
                    for (wt1, wt2, bias_c, dstT) in (
                        (wcq1_t, wcq2_t, fbq_c, QfT),
                        (wck1_t, wck2_t, fbk_c, KfT),
                    ):
                        pss = p1qk.tile([P, G], f32, tag="psqk",
                                        name=f"psqk_{c}_{dstT is KfT}")
                        for si, (wt, srcT) in enumerate(((wt1, xgeT), (wt2, xeeT))):
                            for kc in range(KC):
                                st = (si == 0 and kc == 0)
                                sp_ = (si == 1 and kc == KC - 1)
                                for ih in range(NIH):
                                    nc.tensor.matmul(
                                        pss[:, ih * IH:(ih + 1) * IH],
                                        wt[:, kc, csl],
                                        srcT[:, kc, ih * IH:(ih + 1) * IH],
                                        start=st, stop=sp_)
                        nc.scalar.activation(
                            out=dstT[:, c, :], in_=pss, func=AF.Identity,
                            bias=bias_c[:, c:c + 1], scale=1.0)

            # ================= phase 2: attention =========================
            with tc.tile_pool(name="p2w", bufs=2) as p2w, \
                 tc.tile_pool(name="p2s", bufs=2, space="PSUM") as p2s, \
                 tc.tile_pool(name="p2o", bufs=2, space="PSUM") as p2o:

                for h in range(H):
                    hr = (h % 2) * HD
                    hc = h // 2
                    # oa rows 0:64 accumulate head output; row 64 accumulates
                    # the denominator cA*s2 + cB*d1 (cA via V_st stat column,
                    # cB via the interleaved k=1 matmul below)
                    oa = [p2o.tile([HD + 1, IH], f32, tag="oa", name=f"oa_{h}_{i}") for i in range(NIH)]
                    for jc in range(ICH):
                        et = p2w.tile([P, G], bf16, tag="et")
                        pt_ = p2w.tile([P, G], bf16, tag="pt")
                        sp = p2s.tile([P, G], f32, tag="sp")
                        for ih in range(NIH):
                            nc.tensor.matmul(
                                sp[:, ih * IH:(ih + 1) * IH],
                                KfT[hr:hr + HD, hc, jc * P:(jc + 1) * P],
                                QfT[hr:hr + HD, hc, ih * IH:(ih + 1) * IH],
                                start=True, stop=True)
                        nc.scalar.activation(out=et, in_=sp, func=AF.Exp,
                                             scale=SCALE)
                        nc.vector.tensor_tensor(out=pt_, in0=et, in1=MT[:, jc, :],
                                                op=OP.mult)
                        for ih in range(NIH):
                            nc.tensor.matmul(oa[ih],
                                             V_st[:, jc, h, :],
                                             pt_[:, ih * IH:(ih + 1) * IH],
                                             start=(jc == 0), stop=False,
                                             skip_group_check=(jc > 0))
                        for ih in range(NIH):
                            nc.tensor.matmul(oa[ih][HD:HD + 1, :],
                                             cB_col,
                                             et[:, ih * IH:(ih + 1) * IH],
                                             start=False,
                                             stop=(jc == ICH - 1),
                                             skip_group_check=True)
                    # per-head scale: 1/denominator, then broadcast across the
                    # 64 head rows with a k=1 ones-matmul and apply to OT
                    rr = p2w.tile([1, G], f32, tag="rr", name=f"rr_{h}")
                    rb = p2w.tile([1, G], bf16, tag="rb", name=f"rb_{h}")
                    for ih in range(NIH):
                        isl = slice(ih * IH, (ih + 1) * IH)
                        nc.vector.reciprocal(out=rr[:, isl],
                                             in_=oa[ih][HD:HD + 1, :])
                    nc.vector.tensor_copy(out=rb, in_=rr)
                    for ih in range(NIH):
                        isl = slice(ih * IH, (ih + 1) * IH)
                        nc.vector.tensor_copy(out=OT[hr:hr + HD, hc, isl],
                                              in_=oa[ih][0:HD, :])
                        srt = p2s.tile([P, G], f32, tag="sp",
                                       name=f"srep_{h}_{ih}")
                        srep = srt[0:HD, 0:IH]
                        nc.tensor.matmul(srep, ones_row, rb[:, isl],
                                         start=True, stop=True)
                        nc.vector.tensor_tensor(out=OT[hr:hr + HD, hc, isl],
                                                in0=OT[hr:hr + HD, hc, isl],
                                                in1=srep, op=OP.mult)

            # ============ phase 3: out-proj, residual, LN2, FFN ===========
            pB_cm.__exit__(None, None, None)
            with tc.tile_pool(name="p3", bufs=1) as p3, \
                 tc.tile_pool(name="p3w", bufs=3) as p3w, \
                 tc.tile_pool(name="p3ps", bufs=2, space="PSUM") as p3ps, \
                 tc.tile_pool(name="p3pf", bufs=2, space="PSUM") as p3pf, \
                 tc.tile_pool(name="p3pt", bufs=2, space="PSUM") as p3pt:

                x2T = p3.tile([P, KC, G], bf16, tag="x2T")

                for ic in range(ICH):
                    ps = p3ps.tile([P, D], f32, tag="ps")
                    for dc in range(KC):
                        nc.tensor.matmul(ps,
                                         OT[:, dc, ic * P:(ic + 1) * P],
                                         wo_t[:, dc, :],
                                         start=(dc == 0), stop=(dc == KC - 1))
                    in2c = p3w.tile([P, D], f32, tag="in2c")
                    nc.gpsimd.dma_start(out=in2c, in_=in2b[ic * P:(ic + 1) * P, :])
                    nc.vector.tensor_add(out=h_res[:, ic, :], in0=ps, in1=in2c)
                    x2c = p3w.tile([P, D], bf16, tag="x2c")
                    ln_chunk(h_res[:, ic, :], x2c, p3w)
                    pt = p3pt.tile([P, D], bf16, tag="pt")
                    for kc in range(KC):
                        nc.tensor.transpose(pt[:, kc * P:(kc + 1) * P],
                                            x2c[:, kc * P:(kc + 1) * P], identb)
                    nc.scalar.activation(
                        out=x2T[:, :, ic * P:(ic + 1) * P],
                        in_=pt.rearrange("p (c i) -> p c i", i=P),
                        func=AF.Copy)

                h1g = p3.tile([P, FC, G], bf16, tag="h1g")
                for fc in range(FC):
                    pss = p3pf.tile([P, G], f32, tag="psf", name=f"psf_{fc}")
                    for kc in range(KC):
                        for ih in range(NIH):
                            nc.tensor.matmul(pss[:, ih * IH:(ih + 1) * IH],
                                             w1_t[:, kc, fc * P:(fc + 1) * P],
                                             x2T[:, kc, ih * IH:(ih + 1) * IH],
                                             start=(kc == 0), stop=(kc == KC - 1))
                    nc.scalar.activation(out=h1g[:, fc, :], in_=pss, func=AF.Gelu,
                                         bias=b1_c[:, fc:fc + 1], scale=1.0)
                # FFN2 token-major: out[i, d] = h1^T @ w2, then + h_res + b2
                for ic in range(ICH):
                    ps = p3ps.tile([P, D], f32, tag="ps", name=f"ps2_{ic}")
                    for fc in range(FC):
                        nc.tensor.matmul(ps,
                                         h1g[:, fc, ic * P:(ic + 1) * P],
                                         w2_t[:, fc, :],
                                         start=(fc == 0), stop=(fc == FC - 1))
                    t0 = p3w.tile([P, D], f32, tag="res_t0")
                    nc.vector.tensor_add(out=t0, in0=ps, in1=h_res[:, ic, :])
                    outc = p3w.tile([P, D], f32, tag="outc")
                    nc.gpsimd.tensor_tensor(out=outc, in0=t0, in1=b2_rep,
                                            op=OP.add)
                    nc.scalar.dma_start(out=out_d[ic * P:(ic + 1) * P, :],
                                        in_=outc)

    if split_waits:
        _split_sync_waits(nc, mybir)
    return nc


def _split_sync_waits(nc, mybir, maxw=1):
    """walrus CoreV3 codegen allows only one sem wait per instruction; move
    excess waits onto same-engine nops inserted before the instruction."""
    nid = 0
    for fn in nc.m.functions:
        for blk in fn.blocks:
            orig = list(blk.instructions)
            if not any(i.sync_info and i.sync_info.on_wait and
                       len(i.sync_info.on_wait) > maxw for i in orig):
                continue
            new = []
            for ins in orig:
                si = ins.sync_info
                waits = list(si.on_wait) if si and si.on_wait else []
                if len(waits) > maxw:
                    si.on_wait = waits[:maxw]
                    for k in range(maxw, len(waits), maxw):
                        nop = mybir.InstNoOp(name=f"I-wsplit-{nid}", ins=[], outs=[])
                        nid += 1
                        nop.engine = ins.engine
                        nop.sync_info = mybir.SyncInfo(
                            on_wait=waits[k:k + maxw], on_update=[])
                        new.append(nop)
                new.append(ins)
            blk.instructions = new


def _in_maps(inputs):
    import ml_dtypes
    BF = ml_dtypes.bfloat16
    inp = {k: np.ascontiguousarray(np.asarray(v, np.float32)) for k, v in inputs.items()}
    I64 = np.eye(HD, dtype=np.float32)
    sel2 = np.concatenate([np.zeros((HD, HD), np.float32), I64], 0)
    z64 = np.zeros(HD, np.float32)

    def fold_w(w, f):  # [D, D(out by head)] x [HD, HD] block product
        return np.einsum('ihk,kj->ihj', w.reshape(D, H, HD), f).reshape(D, D)

    def fold_b(b1v, b2v, f, fb):  # fused bias of the head projection [D]
        e = (np.einsum('hk,kj->hj', b1v.reshape(H, HD), f[:HD]) +
             np.einsum('hk,kj->hj', b2v.reshape(H, HD), f[HD:]) + fb[None, :])
        return e.reshape(D)

    maps = []
    for core in range(8):
        b = core % 4
        if core < 4:  # gene branch
            wq1, wq2 = inp["gene_wq"], inp["gene_wq"]
            wk1, wk2 = inp["gene_wk"], inp["gene_wk"]
            bq1 = bq2 = inp["gene_bq"]; bk1 = bk2 = inp["gene_bk"]
            fwq, fbqv, fwk, fbkv = sel2, z64, sel2, z64
            ln1g, ln1b = inp["ln_g1_g"], inp["ln_g1_b"]
            ln2g, ln2b = inp["ln_g2_g"], inp["ln_g2_b"]
            m = dict(
                in1=inp["gene_emb"][b], in2=inp["gene_emb"][b],
                in2b=inp["gene_emb"][b] + inp["gout_b"][None, :],
                wv2=inp["gene_wv"], bv2=inp["gene_bv"],
                wo=inp["gout_w"],
                w1=inp["ffn_g_w1"], b1=inp["ffn_g_b1"],
                w2=inp["ffn_g_w2"], b2=inp["ffn_g_b2"],
                coef=np.array([1.0, 1e-8], np.float32).astype(BF),
            )
        else:  # expr branch
            wq1, wq2 = inp["gene_wq"], inp["expr_wq"]
            wk1, wk2 = inp["gene_wk"], inp["expr_wk"]
            bq1, bq2 = inp["gene_bq"], inp["expr_bq"]
            bk1, bk2 = inp["gene_bk"], inp["expr_bk"]
            fwq, fbqv = inp["fused_wq"], inp["fused_bq"]
            fwk, fbkv = inp["fused_wk"], inp["fused_bk"]
            ln1g, ln1b = inp["ln_e1_g"], inp["ln_e1_b"]
            ln2g, ln2b = inp["ln_e2_g"], inp["ln_e2_b"]
            m = dict(
                in1=inp["gene_emb"][b], in2=inp["expr_emb"][b],
                in2b=inp["expr_emb"][b] + inp["eout_b"][None, :],
                wv2=inp["expr_wv"], bv2=inp["expr_bv"],
                wo=inp["eout_w"],
                w1=inp["ffn_e_w1"], b1=inp["ffn_e_b1"],
                w2=inp["ffn_e_w2"], b2=inp["ffn_e_b2"],
                coef=np.array([0.0, 1.0], np.float32).astype(BF),
            )
        m["MT"] = inp["M"][b].T.astype(BF)
        # fold fused head projection, then LN1 affine into the Q/K weights
        wcq1 = fold_w(wq1, fwq[:HD]); wcq2 = fold_w(wq2, fwq[HD:])
        wck1 = fold_w(wk1, fwk[:HD]); wck2 = fold_w(wk2, fwk[HD:])
        m["fbq"] = (fold_b(bq1, bq2, fwq, fbqv)
                    + ln1b @ wcq1 + ln1b @ wcq2)
        m["fbk"] = (fold_b(bk1, bk2, fwk, fbkv)
                    + ln1b @ wck1 + ln1b @ wck2)
        m["wcq1"] = (ln1g[:, None] * wcq1).astype(BF)
        m["wcq2"] = (ln1g[:, None] * wcq2).astype(BF)
        m["wck1"] = (ln1g[:, None] * wck1).astype(BF)
        m["wck2"] = (ln1g[:, None] * wck2).astype(BF)
        # LN1 affine into V projection
        m["bv2"] = m["bv2"] + ln1b @ m["wv2"]
        m["wv2"] = (ln1g[:, None] * m["wv2"]).astype(BF)
        # LN2 affine into FFN first layer
        m["b1"] = m["b1"] + ln2b @ m["w1"]
        m["w1"] = (ln2g[:, None] * m["w1"]).astype(BF)
        for k in ("wo", "w2"):
            m[k] = m[k].astype(BF)
        maps.append({k: np.ascontiguousarray(v) for k, v in m.items()})
    return maps


def kernel(**inputs):
    from concourse.bass_utils import run_bass_kernel_spmd

    if "nc" not in _cache:
        _cache["nc"] = _build_program()
    nc = _cache["nc"]

    res = run_bass_kernel_spmd(nc, _in_maps(inputs), core_ids=list(range(8)))
    out_gene = np.stack([res.results[c]["out"] for c in range(4)])
    out_expr = np.stack([res.results[c]["out"] for c in range(4, 8)])
    return (out_gene, out_expr)


# revision 20
# speedup vs baseline: 1.2314x; 1.2314x over previous
"""Trainium2 Bass kernel for nn_DeepSCTransformerBlock.

Sharding: 8 cores = 4 batches x 2 branches (gene/expr). Zero collectives.
One SPMD program; the gene branch is expressed through the expr skeleton by
host-side weight folding, and the (s2, d1) normalization coefficients
(cA, cB) are per-core input data:
  gene: scale_i = 1/(1*s2 + 1e-8*d1)   [signed-L1 renorm of softmax*mask]
  expr: scale_i = 1/(0*s2 + 1*d1)      [plain softmax denominator]
where E = exp(S), P = E*M, d1 = colsum(E), s2 = colsum(P); O = scale * (P @ V).

The denominator cA*s2 + cB*d1 is produced directly in PSUM row 64 of the AV
accumulator: V carries an extra column equal to cA (so the AV matmul adds
cA * colsum(P)), and an interleaved k=1 matmul with a cB-filled stationary
accumulates cB * colsum(E) into the same row.

Host-side folds (all exact, fp32):
  - fused head projection: Qf = concat(Q1,Q2) @ fwq + fbq collapses into two
    D x D weights (per-head block product) -> one accumulated projection.
  - LayerNorm affine (gamma/beta) folded into every consumer weight/bias, so
    on-chip LN is just (x - mean) * rstd.
  - out-proj bias bo folded into the residual input (in2b = in2 + bo).
  - M passed pre-transposed ([j, i]) so the mask tile is a straight DMA.

All matmul operands are bf16 (1 PE cycle/row vs 4 for fp32); PSUM accumulates
fp32; the residual stream, LN statistics, and softmax scale math stay fp32.
Activations flow feature-major ([d, tokens]) through the matmul chains; S is
computed transposed ([j, i]) so the AV contraction needs no transposes of the
attention matrix; FFN2 contracts feature-major h1 directly into token-major
output (no final transposes). Bulk weight prefetches ride the gpsimd DMA
queue so they never block the input loads on the sync queue.
"""

import numpy as np

B, G, D, H = 4, 1024, 512, 8
HD = D // H
DFF = 4 * D
P = 128
ICH = G // P      # 8 token chunks
KC = D // P       # 4 feature chunks
FC = DFF // P     # 16 ffn-hidden chunks
NIH = 2           # i halves (free-dim 512 per matmul)
IH = G // NIH     # 512
SCALE = 1.0 / (HD ** 0.5)
EPS = 1e-5

_cache = {}


def _build_program(split_waits=True):
    import contextlib
    import concourse.bass as bass
    import concourse.mybir as mybir
    import concourse.tile as tile
    from concourse.masks import make_identity

    # walrus CoreV3 codegen rejects instructions carrying >1 sem wait at the
    # Tile end-of-kernel drain; split the waits across single-wait nops.
    def _patched_drain_and_barrier(self, tick_clock, wait_clock):
        nc = self.nc
        drain_inst = nc.sync.drain()
        wait_clock.add_sem_waits(
            drain_inst.ins, tile.ScopedClock({None: tick_clock.global_clock})
        )
        si = drain_inst.ins.sync_info
        if si is not None and si.on_wait and len(si.on_wait) > 1:
            waits = list(si.on_wait)
            si.on_wait = waits[:1]
            for i in range(1, len(waits)):
                nop = nc.sync.nop(hint="drain_wait_split", nofuse=True)
                nop.ins.sync_info = mybir.SyncInfo(
                    on_wait=waits[i : i + 1], on_update=[]
                )
        nc.all_engine_barrier()
        assert self.sems is not None
        popped = nc._tile_sem_poison_stack.pop()
        assert popped is self._sem_poison
        nc.clear_and_free_semaphores(list(self.sems.allocated().values()))
        nc.all_engine_barrier()

    tile.TileContext._drain_and_barrier = _patched_drain_and_barrier

    f32 = mybir.dt.float32
    bf16 = mybir.dt.bfloat16
    AF = mybir.ActivationFunctionType
    OP = mybir.AluOpType

    nc = bass.Bass()

    def dram_in(name, shape, dt=f32):
        return nc.dram_tensor(name, list(shape), dt, kind="ExternalInput")

    in1 = dram_in("in1", (G, D))
    in2 = dram_in("in2", (G, D))
    in2b = dram_in("in2b", (G, D))           # in2 + bo (residual base)
    MdT = dram_in("MT", (G, G), bf16)        # pre-transposed mask [j, i]
    wcq1 = dram_in("wcq1", (D, D), bf16)
    wcq2 = dram_in("wcq2", (D, D), bf16)
    wck1 = dram_in("wck1", (D, D), bf16)
    wck2 = dram_in("wck2", (D, D), bf16)
    fbq = dram_in("fbq", (D,))
    fbk = dram_in("fbk", (D,))
    wv2 = dram_in("wv2", (D, D), bf16); bv2 = dram_in("bv2", (D,))
    wo = dram_in("wo", (D, D), bf16)
    w1 = dram_in("w1", (D, DFF), bf16); b1 = dram_in("b1", (DFF,))
    w2 = dram_in("w2", (DFF, D), bf16); b2 = dram_in("b2", (D,))
    coef = dram_in("coef", (2,), bf16)       # [cA, cB]

    out_d = nc.dram_tensor("out", [G, D], f32, kind="ExternalOutput")

    with tile.TileContext(nc) as tc:
        with contextlib.ExitStack() as ctx:
            pc = ctx.enter_context(tc.tile_pool(name="const", bufs=1))
            identb = pc.tile([P, P], bf16, tag="identb")
            make_identity(nc, identb)
            eps_t = pc.tile([P, 1], f32, tag="eps")
            nc.vector.memset(eps_t, EPS)
            cB_col = pc.tile([P, 1], bf16, tag="cB_col")
            nc.sync.dma_start(out=cB_col, in_=coef[None, 1:2].to_broadcast([P, 1]))
            cA_col = pc.tile([P, 1], f32, tag="cA_col")
            nc.gpsimd.dma_start(out=cA_col, in_=coef[None, 0:1].to_broadcast([P, 1]))

            def rep_row(name, vec, n=D):  # [n] dram -> [P, n] replicated rows
                t = pc.tile([P, n], f32, tag=name)
                nc.gpsimd.dma_start(out=t, in_=vec[None, :].to_broadcast([P, n]))
                return t

            bv2_rep = rep_row("bv2_rep", bv2)
            b2_rep = rep_row("b2_rep", b2)

            def col(name, vec, n):  # [n*P] dram -> [P, n] column tile
                t = pc.tile([P, n], f32, tag=name)
                nc.gpsimd.dma_start(out=t, in_=vec.rearrange("(c p) -> p c", p=P))
                return t

            fbq_c = col("fbq_c", fbq, KC); fbk_c = col("fbk_c", fbk, KC)
            b1_c = col("b1_c", b1, FC)
            ones_row = pc.tile([1, HD], bf16, tag="ones_row")
            nc.vector.memset(ones_row, 1.0)

            # long-lived state: residual stream, mask, late weights.
            # bulk prefetches ride the gpsimd DMA queue so the sync queue
            # stays free for the phase-1 input chunk loads.
            pD = ctx.enter_context(tc.tile_pool(name="resid", bufs=1))
            OT = pD.tile([P, KC, G], bf16, tag="OT")        # merged heads, feature-major
            h_res = pD.tile([P, ICH, D], f32, tag="h_res")  # token-major residual
            MT = pD.tile([P, ICH, G], bf16, tag="MT")       # [j-part, jc, i]
            nc.gpsimd.dma_start(out=MT,
                                in_=MdT.rearrange("(jc p) i -> p jc i", p=P))
            wo_t = pD.tile([P, KC, D], bf16, tag="wo_t")
            nc.gpsimd.dma_start(out=wo_t, in_=wo.rearrange("(c p) n -> p c n", p=P))
            w1_t = pD.tile([P, KC, DFF], bf16, tag="w1_t")
            nc.gpsimd.dma_start(out=w1_t, in_=w1.rearrange("(c p) n -> p c n", p=P))
            w2_t = pD.tile([P, FC, D], bf16, tag="w2_t")
            nc.gpsimd.dma_start(out=w2_t, in_=w2.rearrange("(c p) n -> p c n", p=P))

            # long-lived attention operands (packed head layout: head h ->
            # partition rows (h%2)*64, chunk h//2); freed before phase 3
            pB_cm = tc.tile_pool(name="attn_ops", bufs=1)
            pB = pB_cm.__enter__()
            QfT = pB.tile([P, H // 2, G], bf16, tag="QfT")
            KfT = pB.tile([P, H // 2, G], bf16, tag="KfT")
            V_st = pB.tile([P, ICH, H, HD + 1], bf16, tag="V_st")  # [j, jc, h, 65]

            nc.vector.memset(V_st, 1.0)  # V parts overwritten below
            # stat columns <- cA  (AV matmul then adds cA * colsum(P) to row 64)
            statcols = V_st.rearrange("p jc h c -> p (jc h) c")[:, :, HD:HD + 1]
            nc.vector.tensor_scalar_mul(out=statcols, in0=statcols,
                                        scalar1=cA_col)

            def ln_chunk(src_ap, xg_out, wkp):
                """Plain LayerNorm (no affine) of a [P, D] fp32 chunk -> bf16."""
                stats = wkp.tile([P, 6], f32, tag="ln_stats")
                mv = wkp.tile([P, 2], f32, tag="ln_mv")
                nc.vector.bn_stats(out=stats, in_=src_ap)
                nc.vector.bn_aggr(out=mv, in_=stats)
                stdt = wkp.tile([P, 1], f32, tag="ln_std")
                nc.scalar.activation(out=stdt, in_=mv[:, 1:2], func=AF.Sqrt,
                                     bias=eps_t, scale=1.0)
                rstd = wkp.tile([P, 1], f32, tag="ln_rstd")
                nc.vector.reciprocal(out=rstd, in_=stdt)
                nc.vector.tensor_scalar(out=xg_out, in0=src_ap, scalar1=mv[:, 0:1],
                                        scalar2=rstd, op0=OP.subtract, op1=OP.mult)

            # ============ phase 1: LN1, V proj, folded Qf/Kf proj ==========
            with tc.tile_pool(name="p1", bufs=1) as p1, \
                 tc.tile_pool(name="p1w", bufs=2) as p1w, \
                 tc.tile_pool(name="p1ps", bufs=2, space="PSUM") as p1ps, \
                 tc.tile_pool(name="p1qk", bufs=2, space="PSUM") as p1qk, \
                 tc.tile_pool(name="p1pt", bufs=2, space="PSUM") as p1pt:

                xgeT = p1.tile([P, KC, G], bf16, tag="xgeT")
                xeeT = p1.tile([P, KC, G], bf16, tag="xeeT")
                for (src, dstT) in ((in1, xgeT), (in2, xeeT)):
                    for ic in range(ICH):
                        xc = p1w.tile([P, D], f32, tag="ln_in")
                        nc.sync.dma_start(out=xc, in_=src[ic * P:(ic + 1) * P, :])
                        xg = p1w.tile([P, D], bf16, tag="ln_out")
                        ln_chunk(xc, xg, p1w)
                        pt = p1pt.tile([P, D], bf16, tag="pt")
                        for kc in range(KC):
                            nc.tensor.transpose(pt[:, kc * P:(kc + 1) * P],
                                                xg[:, kc * P:(kc + 1) * P], identb)
                        nc.scalar.activation(
                            out=dstT[:, :, ic * P:(ic + 1) * P],
                            in_=pt.rearrange("p (c i) -> p c i", i=P),
                            func=AF.Copy)

                def load_w(name, wd):
                    t = p1.tile([P, KC, D], bf16, tag=name)
                    nc.sync.dma_start(out=t, in_=wd.rearrange("(c p) n -> p c n", p=P))
                    return t

                wcq1_t = load_w("wcq1_t", wcq1)
                wcq2_t = load_w("wcq2_t", wcq2)
                wck1_t = load_w("wck1_t", wck1)
                wck2_t = load_w("wck2_t", wck2)
                wv2_t = load_w("wv2_t", wv2)

                # V projection (token-major) into V_st slots
                for jc in range(ICH):
                    ps = p1ps.tile([P, D], f32, tag="ps")
                    for kc in range(KC):
                        nc.tensor.matmul(ps,
                                         xeeT[:, kc, jc * P:(jc + 1) * P],
                                         wv2_t[:, kc, :],
                                         start=(kc == 0), stop=(kc == KC - 1))
                    nc.vector.tensor_tensor(
                        out=V_st[:, jc, :, 0:HD],
                        in0=ps.rearrange("p (h d) -> p h d", d=HD),
                        in1=bv2_rep.rearrange("p (h d) -> p h d", d=HD),
                        op=OP.add)

                # folded Qf/Kf projections, per head-pair chunk c:
                # QfT[:,c] = Wcq1[:,c]^T @ xgeT + Wcq2[:,c]^T @ xeeT + fbq
                for c in range(KC):
                    csl = slice(c * P, (c + 1) * P)
                    for (wt1, wt2, bias_c, dstT) in (
                        (wcq1_t, wcq2_t, fbq_c, QfT),
                        (wck1_t, wck2_t, fbk_c, KfT),
                    ):
                        pss = p1qk.tile([P, G], f32, tag="psqk",
                                        name=f"psqk_{c}_{dstT is KfT}")
                        for si, (wt, srcT) in enumerate(((wt1, xgeT), (wt2, xeeT))):
                            for kc in range(KC):
                                st = (si == 0 and kc == 0)
                                sp_ = (si == 1 and kc == KC - 1)
                                for ih in range(NIH):
                                    nc.tensor.matmul(
                                        pss[:, ih * IH:(ih + 1) * IH],
                                        wt[:, kc, csl],
                                        srcT[:, kc, ih * IH:(ih + 1) * IH],
                                        start=st, stop=sp_)
                        nc.scalar.activation(
                            out=dstT[:, c, :], in_=pss, func=AF.Identity,
                            bias=bias_c[:, c:c + 1], scale=1.0)

            # ================= phase 2: attention =========================
            with tc.tile_pool(name="p2w", bufs=2) as p2w, \
                 tc.tile_pool(name="p2s", bufs=2, space="PSUM") as p2s, \
                 tc.tile_pool(name="p2r", bufs=2, space="PSUM") as p2r, \
                 tc.tile_pool(name="p2o", bufs=2, space="PSUM") as p2o:

                for h in range(H):
                    hr = (h % 2) * HD
                    hc = h // 2
                    # oa rows 0:64 accumulate head output; row 64 accumulates
                    # the denominator cA*s2 + cB*d1 (cA via V_st stat column,
                    # cB via the interleaved k=1 matmul below)
                    oa = [p2o.tile([HD + 1, IH], f32, tag="oa", name=f"oa_{h}_{i}") for i in range(NIH)]
                    for jc in range(ICH):
                        et = p2w.tile([P, G], bf16, tag="et")
                        pt_ = p2w.tile([P, G], bf16, tag="pt")
                        sp = p2s.tile([P, G], f32, tag="sp")
                        for ih in range(NIH):
                            nc.tensor.matmul(
                                sp[:, ih * IH:(ih + 1) * IH],
                                KfT[hr:hr + HD, hc, jc * P:(jc + 1) * P],
                                QfT[hr:hr + HD, hc, ih * IH:(ih + 1) * IH],
                                start=True, stop=True)
                        nc.scalar.activation(out=et, in_=sp, func=AF.Exp,
                                             scale=SCALE)
                        nc.vector.tensor_tensor(out=pt_, in0=et, in1=MT[:, jc, :],
                                                op=OP.mult)
                        for ih in range(NIH):
                            nc.tensor.matmul(oa[ih],
                                             V_st[:, jc, h, :],
                                             pt_[:, ih * IH:(ih + 1) * IH],
                                             start=(jc == 0), stop=False,
                                             skip_group_check=(jc > 0))
                        for ih in range(NIH):
                            nc.tensor.matmul(oa[ih][HD:HD + 1, :],
                                             cB_col,
                                             et[:, ih * IH:(ih + 1) * IH],
                                             start=False,
                                             stop=(jc == ICH - 1),
                                             skip_group_check=True)
                    # per-head scale: 1/denominator, then broadcast across the
                    # 64 head rows with a k=1 ones-matmul and apply to OT
                    rr = p2w.tile([1, G], f32, tag="rr", name=f"rr_{h}")
                    rb = p2w.tile([1, G], bf16, tag="rb", name=f"rb_{h}")
                    for ih in range(NIH):
                        isl = slice(ih * IH, (ih + 1) * IH)
                        nc.vector.reciprocal(out=rr[:, isl],
                                             in_=oa[ih][HD:HD + 1, :])
                    nc.vector.tensor_copy(out=rb, in_=rr)
                    for ih in range(NIH):
                        isl = slice(ih * IH, (ih + 1) * IH)
                        nc.vector.tensor_copy(out=OT[hr:hr + HD, hc, isl],
                                              in_=oa[ih][0:HD, :])
                        srep = p2r.tile([HD, IH], f32, tag="srep",
                                        name=f"srep_{h}_{ih}")
                        nc.tensor.matmul(srep, ones_row, rb[:, isl],
                                         start=True, stop=True)
                        nc.vector.tensor_tensor(out=OT[hr:hr + HD, hc, isl],
                                                in0=OT[hr:hr + HD, hc, isl],
                                                in1=srep, op=OP.mult)

            # ============ phase 3: out-proj, residual, LN2, FFN ===========
            pB_cm.__exit__(None, None, None)
            with tc.tile_pool(name="p3", bufs=1) as p3, \
                 tc.tile_pool(name="p3w", bufs=3) as p3w, \
                 tc.tile_pool(name="p3ps", bufs=2, space="PSUM") as p3ps, \
                 tc.tile_pool(name="p3pf", bufs=2, space="PSUM") as p3pf, \
                 tc.tile_pool(name="p3pt", bufs=2, space="PSUM") as p3pt:

                x2T = p3.tile([P, KC, G], bf16, tag="x2T")

                for ic in range(ICH):
                    ps = p3ps.tile([P, D], f32, tag="ps")
                    for dc in range(KC):
                        nc.tensor.matmul(ps,
                                         OT[:, dc, ic * P:(ic + 1) * P],
                                         wo_t[:, dc, :],
                                         start=(dc == 0), stop=(dc == KC - 1))
                    in2c = p3w.tile([P, D], f32, tag="in2c")
                    nc.gpsimd.dma_start(out=in2c, in_=in2b[ic * P:(ic + 1) * P, :])
                    nc.vector.tensor_add(out=h_res[:, ic, :], in0=ps, in1=in2c)
                    x2c = p3w.tile([P, D], bf16, tag="x2c")
                    ln_chunk(h_res[:, ic, :], x2c, p3w)
                    pt = p3pt.tile([P, D], bf16, tag="pt")
                    for kc in range(KC):
                        nc.tensor.transpose(pt[:, kc * P:(kc + 1) * P],
                                            x2c[:, kc * P:(kc + 1) * P], identb)
                    nc.scalar.activation(
                        out=x2T[:, :, ic * P:(ic + 1) * P],
                        in_=pt.rearrange("p (c i) -> p c i", i=P),
                        func=AF.Copy)

                h1g = p3.tile([P, FC, G], bf16, tag="h1g")
                for fc in range(FC):
                    pss = p3pf.tile([P, G], f32, tag="psf", name=f"psf_{fc}")
                    for kc in range(KC):
                        for ih in range(NIH):
                            nc.tensor.matmul(pss[:, ih * IH:(ih + 1) * IH],
                                             w1_t[:, kc, fc * P:(fc + 1) * P],
                                             x2T[:, kc, ih * IH:(ih + 1) * IH],
                                             start=(kc == 0), stop=(kc == KC - 1))
                    nc.scalar.activation(out=h1g[:, fc, :], in_=pss, func=AF.Gelu,
                                         bias=b1_c[:, fc:fc + 1], scale=1.0)
                # FFN2 token-major: out[i, d] = h1^T @ w2, then + h_res + b2
                for ic in range(ICH):
                    ps = p3ps.tile([P, D], f32, tag="ps", name=f"ps2_{ic}")
                    for fc in range(FC):
                        nc.tensor.matmul(ps,
                                         h1g[:, fc, ic * P:(ic + 1) * P],
                                         w2_t[:, fc, :],
                                         start=(fc == 0), stop=(fc == FC - 1))
                    t0 = p3w.tile([P, D], f32, tag="res_t0")
                    nc.vector.tensor_add(out=t0, in0=ps, in1=h_res[:, ic, :])
                    outc = p3w.tile([P, D], f32, tag="outc")
                    nc.gpsimd.tensor_tensor(out=outc, in0=t0, in1=b2_rep,
                                            op=OP.add)
                    nc.scalar.dma_start(out=out_d[ic * P:(ic + 1) * P, :],
                                        in_=outc)

    if split_waits:
        _split_sync_waits(nc, mybir)
    return nc


def _split_sync_waits(nc, mybir, maxw=1):
    """walrus CoreV3 codegen allows only one sem wait per instruction; move
    excess waits onto same-engine nops inserted before the instruction."""
    nid = 0
    for fn in nc.m.functions:
        for blk in fn.blocks:
            orig = list(blk.instructions)
            if not any(i.sync_info and i.sync_info.on_wait and
                       len(i.sync_info.on_wait) > maxw for i in orig):
                continue
            new = []
            for ins in orig:
                si = ins.sync_info
                waits = list(si.on_wait) if si and si.on_wait else []
                if len(waits) > maxw:
                    si.on_wait = waits[:maxw]
                    for k in range(maxw, len(waits), maxw):
                        nop = mybir.InstNoOp(name=f"I-wsplit-{nid}", ins=[], outs=[])
                        nid += 1
                        nop.engine = ins.engine
                        nop.sync_info = mybir.SyncInfo(
                            on_wait=waits[k:k + maxw], on_update=[])
                        new.append(nop)
                new.append(ins)
            blk.instructions = new


def _in_maps(inputs):
    import ml_dtypes
    BF = ml_dtypes.bfloat16
    inp = {k: np.ascontiguousarray(np.asarray(v, np.float32)) for k, v in inputs.items()}
    I64 = np.eye(HD, dtype=np.float32)
    sel2 = np.concatenate([np.zeros((HD, HD), np.float32), I64], 0)
    z64 = np.zeros(HD, np.float32)

    def fold_w(w, f):  # [D, D(out by head)] x [HD, HD] block product
        return np.einsum('ihk,kj->ihj', w.reshape(D, H, HD), f).reshape(D, D)

    def fold_b(b1v, b2v, f, fb):  # fused bias of the head projection [D]
        e = (np.einsum('hk,kj->hj', b1v.reshape(H, HD), f[:HD]) +
             np.einsum('hk,kj->hj', b2v.reshape(H, HD), f[HD:]) + fb[None, :])
        return e.reshape(D)

    maps = []
    for core in range(8):
        b = core % 4
        if core < 4:  # gene branch
            wq1, wq2 = inp["gene_wq"], inp["gene_wq"]
            wk1, wk2 = inp["gene_wk"], inp["gene_wk"]
            bq1 = bq2 = inp["gene_bq"]; bk1 = bk2 = inp["gene_bk"]
            fwq, fbqv, fwk, fbkv = sel2, z64, sel2, z64
            ln1g, ln1b = inp["ln_g1_g"], inp["ln_g1_b"]
            ln2g, ln2b = inp["ln_g2_g"], inp["ln_g2_b"]
            m = dict(
                in1=inp["gene_emb"][b], in2=inp["gene_emb"][b],
                in2b=inp["gene_emb"][b] + inp["gout_b"][None, :],
                wv2=inp["gene_wv"], bv2=inp["gene_bv"],
                wo=inp["gout_w"],
                w1=inp["ffn_g_w1"], b1=inp["ffn_g_b1"],
                w2=inp["ffn_g_w2"], b2=inp["ffn_g_b2"],
                coef=np.array([1.0, 1e-8], np.float32).astype(BF),
            )
        else:  # expr branch
            wq1, wq2 = inp["gene_wq"], inp["expr_wq"]
            wk1, wk2 = inp["gene_wk"], inp["expr_wk"]
            bq1, bq2 = inp["gene_bq"], inp["expr_bq"]
            bk1, bk2 = inp["gene_bk"], inp["expr_bk"]
            fwq, fbqv = inp["fused_wq"], inp["fused_bq"]
            fwk, fbkv = inp["fused_wk"], inp["fused_bk"]
            ln1g, ln1b = inp["ln_e1_g"], inp["ln_e1_b"]
            ln2g, ln2b = inp["ln_e2_g"], inp["ln_e2_b"]
            m = dict(
                in1=inp["gene_emb"][b], in2=inp["expr_emb"][b],
                in2b=inp["expr_emb"][b] + inp["eout_b"][None, :],
                wv2=inp["expr_wv"], bv2=inp["expr_bv"],
                wo=inp["eout_w"],
                w1=inp["ffn_e_w1"], b1=inp["ffn_e_b1"],
                w2=inp["ffn_e_w2"], b2=inp["ffn_e_b2"],
                coef=np.array([0.0, 1.0], np.float32).astype(BF),
            )
        m["MT"] = inp["M"][b].T.astype(BF)
        # fold fused head projection, then LN1 affine into the Q/K weights
        wcq1 = fold_w(wq1, fwq[:HD]); wcq2 = fold_w(wq2, fwq[HD:])
        wck1 = fold_w(wk1, fwk[:HD]); wck2 = fold_w(wk2, fwk[HD:])
        m["fbq"] = (fold_b(bq1, bq2, fwq, fbqv)
                    + ln1b @ wcq1 + ln1b @ wcq2)
        m["fbk"] = (fold_b(bk1, bk2, fwk, fbkv)
                    + ln1b @ wck1 + ln1b @ wck2)
        m["wcq1"] = (ln1g[:, None] * wcq1).astype(BF)
        m["wcq2"] = (ln1g[:, None] * wcq2).astype(BF)
        m["wck1"] = (ln1g[:, None] * wck1).astype(BF)
        m["wck2"] = (ln1g[:, None] * wck2).astype(BF)
        # LN1 affine into V projection
        m["bv2"] = m["bv2"] + ln1b @ m["wv2"]
        m["wv2"] = (ln1g[:, None] * m["wv2"]).astype(BF)
        # LN2 affine into FFN first layer
        m["b1"] = m["b1"] + ln2b @ m["w1"]
        m["w1"] = (ln2g[:, None] * m["w1"]).astype(BF)
        for k in ("wo", "w2"):
            m[k] = m[k].astype(BF)
        maps.append({k: np.ascontiguousarray(v) for k, v in m.items()})
    return maps


def kernel(**inputs):
    from concourse.bass_utils import run_bass_kernel_spmd

    if "nc" not in _cache:
        _cache["nc"] = _build_program()
    nc = _cache["nc"]

    res = run_bass_kernel_spmd(nc, _in_maps(inputs), core_ids=list(range(8)))
    out_gene = np.stack([res.results[c]["out"] for c in range(4)])
    out_expr = np.stack([res.results[c]["out"] for c in range(4, 8)])
    return (out_gene, out_expr)
